# revision 13
# baseline (speedup 1.0000x reference)
"""Trainium2 Bass kernel for nn_BasicBlock (binarized 3x3 conv + BN + ReLU).

Reference computation (NHWC, f32):
    a   = ste_sign(x + bias1)            # +-1, sign(0)=+1
    qk  = ste_sign(kernel)               # +-1
    y   = conv2d(a, qk, SAME, stride 1)  # (32,56,56,256)
    y   = (y - mean) * rsqrt(var+eps) + beta
    out = relu(y + bias2)

Strategy (v2, fp8 DoubleRow):
  - Data-parallel over batch: 8 cores x 4 images, no collectives.
  - Operands are exactly +-1, exact in fp8e4; fp32 PSUM accumulation keeps
    integer conv sums (|y| <= 2304) bit-exact. DoubleRow packs both
    128-channel Cin halves into one matmul at 2 MACs/cell/cycle.
  - Per core pipeline, per image:
      load 8-row groups [112 part, 4 px, 256ch] (4KB/partition descriptors)
      -> PE transpose f32 -> ScalarE Sign(x+bias1) -> fp8 +-1 into a
      zero-padded 58-wide channel-major image buffer
      -> conv: weights-stationary fp8 DoubleRow matmuls, psum [co,464px]
         (8 output rows per group, 9 taps accumulated, 2 Cout tiles)
      -> VectorE BN affine (y*s + t, per-partition scale/shift)
      -> PE transpose back to [px, co] per 2-row tile
      -> VectorE fused relu + PSUM evacuation into a per-image staging
         buffer -> 2 large NHWC stores per image via GPSIMD (SWDGE).
  - Host precomputes constants only: sign(kernel) in fp8 DoubleRow layout,
    folded BN scale/shift, and a 1-ulp nudge of x where fl(x+bias1)==0 so
    device Sign (sign(0)=0) matches ste_sign (sign(0)=+1).
"""

import numpy as np
import ml_dtypes

import concourse.bass as bass
import concourse.mybir as mybir
import concourse.tile as tile
from concourse import bacc
from concourse.bass_utils import run_bass_kernel_spmd
from concourse.masks import make_identity
from concourse.tile_rust import add_dep_helper

# Problem shape (hardcoded per contract).
B, H, W, CIN, COUT = 32, 56, 56, 256, 256
N_CORES = 8
IMG = B // N_CORES          # images per core
EPS = 1e-3

P = 128
WPAD = 58                   # padded row width (56 + 2)
ROWS = 59                   # 1 top pad + 56 real + 1 bottom pad + slack
AFREE = 3424                # ROWS*WPAD=3422 padded to %16 for DoubleRow APs
RP = H // 2                 # 28 row-pairs per image
G8 = H // 8                 # 7 eight-row groups per image
NPX = 8 * WPAD              # 464 psum pixels per conv group

F32 = mybir.dt.float32
FP8 = mybir.dt.float8e4

AluOp = mybir.AluOpType


def _build_program():
    nc = bacc.Bacc(
        "TRN2",
        target_bir_lowering=False,
        debug=False,
        enable_asserts=False,
        num_devices=N_CORES,
    )

    x_ap = nc.dram_tensor("x", (IMG, H, W, CIN), F32, kind="ExternalInput").ap()
    w_ap = nc.dram_tensor("wq", (P, 9, 2, 2, P), FP8, kind="ExternalInput").ap()
    b1_ap = nc.dram_tensor("b1", (2, P), F32, kind="ExternalInput").ap()
    s_ap = nc.dram_tensor("s", (2, P), F32, kind="ExternalInput").ap()
    t_ap = nc.dram_tensor("t", (2, P), F32, kind="ExternalInput").ap()
    out_ap = nc.dram_tensor("out", (IMG, H, W, COUT), F32, kind="ExternalOutput").ap()

    x_flat = x_ap.rearrange("b h w c -> b (h w) c")

    # Chain every PE instruction in program order. Repeated-stationary
    # matmuls run with ldweights=False (array reuse), which is only safe if
    # no other PE op can be scheduled between them and their weight load.
    pe_prev = [None]

    def pe(bi):
        if pe_prev[0] is not None:
            add_dep_helper(bi.ins, pe_prev[0], sync=False, reason="pe order")
        pe_prev[0] = bi.ins
        return bi

    with tile.TileContext(nc) as tc:
        with (
            tc.tile_pool(name="const", bufs=1) as const_pool,
            tc.tile_pool(name="xin", bufs=4) as x_pool,
            tc.tile_pool(name="ybn", bufs=16) as y_pool,
            tc.tile_pool(name="pst", bufs=2, space="PSUM") as pst_pool,
            tc.tile_pool(name="pso", bufs=4, space="PSUM") as pso_pool,
            tc.tile_pool(name="psu", bufs=2, space="PSUM") as psu_pool,
        ):
            w_sb = const_pool.tile([P, 9, 2, 2, P], FP8)
            nc.sync.dma_start(w_sb[:], w_ap)
            b1_sb = const_pool.tile([P, 2], F32)
            nc.sync.dma_start(b1_sb[:], b1_ap.rearrange("t p -> p t"))
            s_sb = const_pool.tile([P, 2], F32)
            nc.sync.dma_start(s_sb[:], s_ap.rearrange("t p -> p t"))
            t_sb = const_pool.tile([P, 2], F32)
            nc.sync.dma_start(t_sb[:], t_ap.rearrange("t p -> p t"))
            ident = const_pool.tile([P, P], F32)
            make_identity(nc, ident[:])

            # Double-buffered fp8 padded activation buffers + f32 output stage.
            acts = [
                const_pool.tile([P, 2, AFREE], FP8, name=f"act{i}") for i in range(2)
            ]
            ubig = [
                const_pool.tile([P, RP, COUT], F32, name=f"ubig{i}") for i in range(2)
            ]
            nc.gpsimd.memset(acts[0][:], 0.0)
            nc.gpsimd.memset(acts[1][:], 0.0)

            for b in range(IMG):
                slot = b % 2
                act = acts[slot]
                ub = ubig[slot]

                # ---- load + transpose + binarize (8-row groups) ----
                for g in range(G8):
                    xt = x_pool.tile([112, 4, CIN], F32)
                    nc.sync.dma_start(
                        xt[:],
                        x_flat[b, 448 * g : 448 * (g + 1), :].rearrange(
                            "(p j) c -> p j c", p=112
                        ),
                    )
                    for j in range(4):
                        for ci in range(2):
                            pt = pst_pool.tile([P, 112], F32)
                            pe(nc.tensor.transpose(
                                pt[:],
                                xt[:, j, ci * P : (ci + 1) * P],
                                ident[:112, :112],
                            ))
                            # px = 4p + j -> row r = px//56 (8 rows), col 4q+j.
                            # dest: padded rows 8g+1..8g+8, cols 2+j step 4.
                            base = (8 * g + 1) * WPAD + 2 + j
                            dest = (
                                act[:, ci, base : base + 8 * WPAD]
                                .rearrange("p (r w) -> p r w", w=WPAD)[:, :, 0:56]
                                .rearrange("p r (q x) -> p r q x", x=4)[:, :, :, 0]
                            )
                            nc.scalar.activation(
                                dest,
                                pt.rearrange("p (r q) -> p r q", q=14),
                                mybir.ActivationFunctionType.Sign,
                                bias=b1_sb[:, ci : ci + 1],
                                scale=1.0,
                            )

                # ---- conv (fp8 DoubleRow, weights stationary) + BN affine ----
                y_tiles = {}
                for co in range(2):
                    for block in ((0, 1, 2, 3), (4, 5, 6)):
                        pm = {
                            m: pso_pool.tile([P, NPX], F32, name="pm", tag="pm")
                            for m in block
                        }
                        for tap in range(9):
                            dh, dw = tap // 3, tap % 3
                            for mi, m in enumerate(block):
                                rbase = (8 * m + dh) * WPAD + dw
                                inst = pe(nc.tensor.matmul(
                                    pm[m][:],
                                    w_sb[:, tap, co],
                                    act[:, :, rbase : rbase + NPX],
                                    start=(tap == 0),
                                    stop=(tap == 8),
                                    perf_mode=mybir.MatmulPerfMode.DoubleRow,
                                ))
                                if mi > 0:
                                    inst.ins.ldweights = False
                        for m in block:
                            y = y_pool.tile([P, NPX], F32, name="y", tag="y")
                            # y = conv * scale + shift   (per-partition co consts)
                            nc.vector.tensor_scalar(
                                y[:], pm[m][:],
                                s_sb[:, co : co + 1], t_sb[:, co : co + 1],
                                op0=AluOp.mult, op1=AluOp.add,
                            )
                            y_tiles[(co, m)] = y

                # ---- transpose back to [px, co], fused relu, stage, store ----
                for m in range(G8):
                    for r in range(4):
                        k = 4 * m + r
                        pu = psu_pool.tile([116, COUT], F32)
                        for co in range(2):
                            pe(nc.tensor.matmul(
                                pu[:, co * P : (co + 1) * P],
                                y_tiles[(co, m)][:, 116 * r : 116 * r + 116],
                                ident[:, :P],
                                is_transpose=True,
                                start=(co == 0),
                                stop=(co == 1),
                            ))
                        # partitions: 0 pad | 1..56 row 2k | 57,58 pad |
                        # 59..114 row 2k+1 | 115 pad
                        nc.vector.tensor_scalar(
                            ub[:116, k, :], pu[:], 0.0, None, op0=AluOp.max
                        )

                ev = out_ap[b].rearrange("(k two) w c -> w two k c", two=2)
                nc.gpsimd.dma_start(ev[:, 0], ub[1:57])
                nc.gpsimd.dma_start(ev[:, 1], ub[59:115])

    nc.compile()
    return nc


_NC_CACHE = None


def _get_nc():
    global _NC_CACHE
    if _NC_CACHE is None:
        _NC_CACHE = _build_program()
    return _NC_CACHE


def _prep_inputs(x, bias1, kernel, bn_beta, bn_mean, bn_var, bias2):
    x = np.asarray(x, dtype=np.float32)
    bias1 = np.asarray(bias1, dtype=np.float32)
    kernel = np.asarray(kernel, dtype=np.float32)
    bn_beta = np.asarray(bn_beta, dtype=np.float32)
    bn_mean = np.asarray(bn_mean, dtype=np.float32)
    bn_var = np.asarray(bn_var, dtype=np.float32)
    bias2 = np.asarray(bias2, dtype=np.float32).reshape(-1)

    # Device computes sign(fl(x + b)) with sign(0)=0; the reference wants
    # sign(0)=+1. Nudge x by 1 ulp wherever fl(x+b) == 0 exactly (x is only
    # consumed through this sign).
    z = x + bias1
    if np.any(z == 0.0):
        x = np.where(z == 0.0, np.nextafter(x, np.float32(np.inf)), x)

    # Weights: ste_sign with sign(0)=+1, exact in fp8e4.
    # [kh,kw,ci,co] -> [ki, tap, co_t, o(ci half), co] (DoubleRow pairing
    # puts ci = o*128 + ki, matching the act buffer's [ki, ci_t, px] layout).
    wq = np.where(kernel >= 0, np.float32(1.0), np.float32(-1.0))
    wq = wq.reshape(9, 2, P, 2, P).transpose(2, 0, 3, 1, 4)
    wq = np.ascontiguousarray(wq).astype(ml_dtypes.float8_e4m3)

    s = (1.0 / np.sqrt(bn_var + np.float32(EPS))).astype(np.float32)
    t = (bn_beta - bn_mean * s + bias2).astype(np.float32)
    b1 = np.ascontiguousarray(bias1.reshape(2, P)).astype(np.float32)

    in_maps = []
    for c in range(N_CORES):
        in_maps.append(
            {
                "x": np.ascontiguousarray(x[c * IMG : (c + 1) * IMG]),
                "wq": wq,
                "b1": b1,
                "s": np.ascontiguousarray(s.reshape(2, P)),
                "t": np.ascontiguousarray(t.reshape(2, P)),
            }
        )
    return in_maps


def _ensure_ntff_hook():
    """This container ships the NTFF profiling machinery but not the
    ``antenv.axon_hooks`` shim module bass_utils imports it through;
    synthesize it so trace=True can capture HW exec times."""
    import sys
    import types

    if "antenv.axon_hooks" in sys.modules:
        return
    import antenv
    from trn_agent_boot.trn_boot import _ntff_profile_via_ctypes

    hook = _ntff_profile_via_ctypes("/opt/axon/libaxon_pjrt.so")
    mod = types.ModuleType("antenv.axon_hooks")
    mod.get_axon_ntff_profile_hook = lambda: hook
    mod.set_axon_ntff_profile_hook = lambda h: None
    sys.modules["antenv.axon_hooks"] = mod
    antenv.axon_hooks = mod


def run(inputs: dict, trace: bool = False):
    """Run the SPMD kernel. Returns (out, exec_time_ns or None)."""
    nc = _get_nc()
    in_maps = _prep_inputs(**inputs)
    if trace:
        try:
            _ensure_ntff_hook()
        except Exception as e:  # degrade to untraced run
            print(f"ntff hook unavailable: {e}")
    res = run_bass_kernel_spmd(
        nc, in_maps, core_ids=list(range(N_CORES)), trace=trace
    )
    out = np.concatenate([r["out"] for r in res.results], axis=0)
    return out, res.exec_time_ns


def kernel(**inputs) -> np.ndarray:
    out, _ = run(inputs, trace=False)
    return out


# revision 17
# speedup vs baseline: 1.2872x; 1.2872x over previous
"""Trainium2 Bass kernel for nn_BasicBlock (binarized 3x3 conv + BN + ReLU).

Reference computation (NHWC, f32):
    a   = ste_sign(x + bias1)            # +-1, sign(0)=+1
    qk  = ste_sign(kernel)               # +-1
    y   = conv2d(a, qk, SAME, stride 1)  # (32,56,56,256)
    y   = (y - mean) * rsqrt(var+eps) + beta
    out = relu(y + bias2)

Strategy (v2, fp8 DoubleRow):
  - Data-parallel over batch: 8 cores x 4 images, no collectives.
  - Operands are exactly +-1, exact in fp8e4; fp32 PSUM accumulation keeps
    integer conv sums (|y| <= 2304) bit-exact. DoubleRow packs both
    128-channel Cin halves into one matmul at 2 MACs/cell/cycle.
  - Per core pipeline, per image:
      load 8-row groups [112 part, 4 px, 256ch] (4KB/partition descriptors)
      -> PE transpose f32 -> ScalarE Sign(x+bias1) -> fp8 +-1 into a
      zero-padded 58-wide channel-major image buffer
      -> conv: weights-stationary fp8 DoubleRow matmuls, psum [co,464px]
         (8 output rows per group, 9 taps accumulated, 2 Cout tiles)
      -> VectorE BN affine (y*s + t, per-partition scale/shift)
      -> PE transpose back to [px, co] per 2-row tile
      -> VectorE fused relu + PSUM evacuation into a per-image staging
         buffer -> 2 large NHWC stores per image via GPSIMD (SWDGE).
  - Host precomputes constants only: sign(kernel) in fp8 DoubleRow layout,
    folded BN scale/shift, and a 1-ulp nudge of x where fl(x+bias1)==0 so
    device Sign (sign(0)=0) matches ste_sign (sign(0)=+1).
"""

import numpy as np
import ml_dtypes

import concourse.bass as bass
import concourse.mybir as mybir
import concourse.tile as tile
from concourse import bacc
from concourse.bass_utils import run_bass_kernel_spmd
from concourse.masks import make_identity
from concourse.tile_rust import add_dep_helper

# Problem shape (hardcoded per contract).
B, H, W, CIN, COUT = 32, 56, 56, 256, 256
N_CORES = 8
IMG = B // N_CORES          # images per core
EPS = 1e-3

P = 128
WPAD = 58                   # padded row width (56 + 2)
ROWS = 59                   # 1 top pad + 56 real + 1 bottom pad + slack
AFREE = 3424                # ROWS*WPAD=3422 padded to %16 for DoubleRow APs
RP = H // 2                 # 28 row-pairs per image
G8 = H // 8                 # 7 eight-row groups per image
NPX = 8 * WPAD              # 464 psum pixels per conv group

F32 = mybir.dt.float32
FP8 = mybir.dt.float8e4

AluOp = mybir.AluOpType


def _build_program():
    nc = bacc.Bacc(
        "TRN2",
        target_bir_lowering=False,
        debug=False,
        enable_asserts=False,
        num_devices=N_CORES,
    )

    x_ap = nc.dram_tensor("x", (IMG, H, W, CIN), F32, kind="ExternalInput").ap()
    w_ap = nc.dram_tensor("wq", (P, 9, 2, 2, P), FP8, kind="ExternalInput").ap()
    b1_ap = nc.dram_tensor("b1", (2, P), F32, kind="ExternalInput").ap()
    s_ap = nc.dram_tensor("s", (2, P), F32, kind="ExternalInput").ap()
    t_ap = nc.dram_tensor("t", (2, P), F32, kind="ExternalInput").ap()
    out_ap = nc.dram_tensor("out", (IMG, H, W, COUT), F32, kind="ExternalOutput").ap()

    x_flat = x_ap.rearrange("b h w c -> b (h w) c")

    with tile.TileContext(nc) as tc:
        with (
            tc.tile_pool(name="const", bufs=1) as const_pool,
            tc.tile_pool(name="xin", bufs=4) as x_pool,
            tc.tile_pool(name="ybn", bufs=16) as y_pool,
            tc.tile_pool(name="pst", bufs=2, space="PSUM") as pst_pool,
            tc.tile_pool(name="pso", bufs=4, space="PSUM") as pso_pool,
            tc.tile_pool(name="psu", bufs=2, space="PSUM") as psu_pool,
        ):
            w_sb = const_pool.tile([P, 9, 2, 2, P], FP8)
            nc.sync.dma_start(w_sb[:], w_ap)
            b1_sb = const_pool.tile([P, 2], F32)
            nc.sync.dma_start(b1_sb[:], b1_ap.rearrange("t p -> p t"))
            s_sb = const_pool.tile([P, 2], F32)
            nc.sync.dma_start(s_sb[:], s_ap.rearrange("t p -> p t"))
            t_sb = const_pool.tile([P, 2], F32)
            nc.sync.dma_start(t_sb[:], t_ap.rearrange("t p -> p t"))
            ident = const_pool.tile([P, P], F32)
            make_identity(nc, ident[:])

            # Double-buffered fp8 padded activation buffers + f32 output stage.
            acts = [
                const_pool.tile([P, 2, AFREE], FP8, name=f"act{i}") for i in range(2)
            ]
            ubig = [
                const_pool.tile([P, RP, COUT], F32, name=f"ubig{i}") for i in range(2)
            ]
            nc.gpsimd.memset(acts[0][:], 0.0)
            nc.gpsimd.memset(acts[1][:], 0.0)

            for b in range(IMG):
                slot = b % 2
                act = acts[slot]
                ub = ubig[slot]

                # ---- load + transpose + binarize (8-row groups) ----
                for g in range(G8):
                    xt = x_pool.tile([112, 4, CIN], F32)
                    nc.sync.dma_start(
                        xt[:],
                        x_flat[b, 448 * g : 448 * (g + 1), :].rearrange(
                            "(p j) c -> p j c", p=112
                        ),
                    )
                    for j in range(4):
                        for ci in range(2):
                            pt = pst_pool.tile([P, 112], F32)
                            nc.tensor.transpose(
                                pt[:],
                                xt[:, j, ci * P : (ci + 1) * P],
                                ident[:112, :112],
                            )
                            # px = 4p + j -> row r = px//56 (8 rows), col 4q+j.
                            # dest: padded rows 8g+1..8g+8, cols 2+j step 4.
                            base = (8 * g + 1) * WPAD + 2 + j
                            dest = (
                                act[:, ci, base : base + 8 * WPAD]
                                .rearrange("p (r w) -> p r w", w=WPAD)[:, :, 0:56]
                                .rearrange("p r (q x) -> p r q x", x=4)[:, :, :, 0]
                            )
                            nc.scalar.activation(
                                dest,
                                pt.rearrange("p (r q) -> p r q", q=14),
                                mybir.ActivationFunctionType.Sign,
                                bias=b1_sb[:, ci : ci + 1],
                                scale=1.0,
                            )

                # ---- conv (fp8 DoubleRow, weights stationary) + BN affine ----
                y_tiles = {}
                for co in range(2):
                    for block in ((0, 1, 2, 3), (4, 5, 6)):
                        pm = {
                            m: pso_pool.tile([P, NPX], F32, name="pm", tag="pm")
                            for m in block
                        }
                        for tap in range(9):
                            dh, dw = tap // 3, tap % 3
                            # One explicit weight load per tap group; the
                            # matmuls reuse the loaded array (ldweights=False)
                            # and are chained so nothing slips in between.
                            prev = nc.tensor.ldweights(
                                w_sb[:, tap, co],
                                perf_mode=mybir.MatmulPerfMode.DoubleRow,
                            ).ins
                            for m in block:
                                rbase = (8 * m + dh) * WPAD + dw
                                inst = nc.tensor.matmul(
                                    pm[m][:],
                                    w_sb[:, tap, co],
                                    act[:, :, rbase : rbase + NPX],
                                    start=(tap == 0),
                                    stop=(tap == 8),
                                    perf_mode=mybir.MatmulPerfMode.DoubleRow,
                                )
                                inst.ins.ldweights = False
                                add_dep_helper(
                                    inst.ins, prev, sync=False, reason="ldw group"
                                )
                                prev = inst.ins
                        for m in block:
                            y = y_pool.tile([P, NPX], F32, name="y", tag="y")
                            # y = conv * scale + shift   (per-partition co consts)
                            nc.vector.tensor_scalar(
                                y[:], pm[m][:],
                                s_sb[:, co : co + 1], t_sb[:, co : co + 1],
                                op0=AluOp.mult, op1=AluOp.add,
                            )
                            y_tiles[(co, m)] = y

                # ---- transpose back to [px, co], fused relu, stage, store ----
                for m in range(G8):
                    for r in range(4):
                        k = 4 * m + r
                        pu = psu_pool.tile([116, COUT], F32)
                        for co in range(2):
                            nc.tensor.matmul(
                                pu[:, co * P : (co + 1) * P],
                                y_tiles[(co, m)][:, 116 * r : 116 * r + 116],
                                ident[:, :P],
                                is_transpose=True,
                                start=(co == 0),
                                stop=(co == 1),
                            )
                        # partitions: 0 pad | 1..56 row 2k | 57,58 pad |
                        # 59..114 row 2k+1 | 115 pad
                        nc.vector.tensor_scalar(
                            ub[:116, k, :], pu[:], 0.0, None, op0=AluOp.max
                        )

                ev = out_ap[b].rearrange("(k two) w c -> w two k c", two=2)
                nc.gpsimd.dma_start(ev[:, 0], ub[1:57])
                nc.gpsimd.dma_start(ev[:, 1], ub[59:115])

    nc.compile()
    return nc


_NC_CACHE = None


def _get_nc():
    global _NC_CACHE
    if _NC_CACHE is None:
        _NC_CACHE = _build_program()
    return _NC_CACHE


def _prep_inputs(x, bias1, kernel, bn_beta, bn_mean, bn_var, bias2):
    x = np.asarray(x, dtype=np.float32)
    bias1 = np.asarray(bias1, dtype=np.float32)
    kernel = np.asarray(kernel, dtype=np.float32)
    bn_beta = np.asarray(bn_beta, dtype=np.float32)
    bn_mean = np.asarray(bn_mean, dtype=np.float32)
    bn_var = np.asarray(bn_var, dtype=np.float32)
    bias2 = np.asarray(bias2, dtype=np.float32).reshape(-1)

    # Device computes sign(fl(x + b)) with sign(0)=0; the reference wants
    # sign(0)=+1. Nudge x by 1 ulp wherever fl(x+b) == 0 exactly (x is only
    # consumed through this sign).
    z = x + bias1
    if np.any(z == 0.0):
        x = np.where(z == 0.0, np.nextafter(x, np.float32(np.inf)), x)

    # Weights: ste_sign with sign(0)=+1, exact in fp8e4.
    # [kh,kw,ci,co] -> [ki, tap, co_t, o(ci half), co] (DoubleRow pairing
    # puts ci = o*128 + ki, matching the act buffer's [ki, ci_t, px] layout).
    wq = np.where(kernel >= 0, np.float32(1.0), np.float32(-1.0))
    wq = wq.reshape(9, 2, P, 2, P).transpose(2, 0, 3, 1, 4)
    wq = np.ascontiguousarray(wq).astype(ml_dtypes.float8_e4m3)

    s = (1.0 / np.sqrt(bn_var + np.float32(EPS))).astype(np.float32)
    t = (bn_beta - bn_mean * s + bias2).astype(np.float32)
    b1 = np.ascontiguousarray(bias1.reshape(2, P)).astype(np.float32)

    in_maps = []
    for c in range(N_CORES):
        in_maps.append(
            {
                "x": np.ascontiguousarray(x[c * IMG : (c + 1) * IMG]),
                "wq": wq,
                "b1": b1,
                "s": np.ascontiguousarray(s.reshape(2, P)),
                "t": np.ascontiguousarray(t.reshape(2, P)),
            }
        )
    return in_maps


def _ensure_ntff_hook():
    """This container ships the NTFF profiling machinery but not the
    ``antenv.axon_hooks`` shim module bass_utils imports it through;
    synthesize it so trace=True can capture HW exec times."""
    import sys
    import types

    if "antenv.axon_hooks" in sys.modules:
        return
    import antenv
    from trn_agent_boot.trn_boot import _ntff_profile_via_ctypes

    hook = _ntff_profile_via_ctypes("/opt/axon/libaxon_pjrt.so")
    mod = types.ModuleType("antenv.axon_hooks")
    mod.get_axon_ntff_profile_hook = lambda: hook
    mod.set_axon_ntff_profile_hook = lambda h: None
    sys.modules["antenv.axon_hooks"] = mod
    antenv.axon_hooks = mod


def run(inputs: dict, trace: bool = False):
    """Run the SPMD kernel. Returns (out, exec_time_ns or None)."""
    nc = _get_nc()
    in_maps = _prep_inputs(**inputs)
    if trace:
        try:
            _ensure_ntff_hook()
        except Exception as e:  # degrade to untraced run
            print(f"ntff hook unavailable: {e}")
    res = run_bass_kernel_spmd(
        nc, in_maps, core_ids=list(range(N_CORES)), trace=trace
    )
    out = np.concatenate([r["out"] for r in res.results], axis=0)
    return out, res.exec_time_ns


def kernel(**inputs) -> np.ndarray:
    out, _ = run(inputs, trace=False)
    return out


# revision 21
# speedup vs baseline: 1.3525x; 1.0507x over previous
"""Trainium2 Bass kernel for nn_BasicBlock (binarized 3x3 conv + BN + ReLU).

Reference computation (NHWC, f32):
    a   = ste_sign(x + bias1)            # +-1, sign(0)=+1
    qk  = ste_sign(kernel)               # +-1
    y   = conv2d(a, qk, SAME, stride 1)  # (32,56,56,256)
    y   = (y - mean) * rsqrt(var+eps) + beta
    out = relu(y + bias2)

Strategy (v2, fp8 DoubleRow):
  - Data-parallel over batch: 8 cores x 4 images, no collectives.
  - Operands are exactly +-1, exact in fp8e4; fp32 PSUM accumulation keeps
    integer conv sums (|y| <= 2304) bit-exact. DoubleRow packs both
    128-channel Cin halves into one matmul at 2 MACs/cell/cycle.
  - Per core pipeline, per image:
      load 8-row groups [112 part, 4 px, 256ch] (4KB/partition descriptors)
      -> PE transpose f32 -> ScalarE Sign(x+bias1) -> fp8 +-1 into a
      zero-padded 58-wide channel-major image buffer
      -> conv: weights-stationary fp8 DoubleRow matmuls, psum [co,464px]
         (8 output rows per group, 9 taps accumulated, 2 Cout tiles)
      -> VectorE BN affine (y*s + t, per-partition scale/shift)
      -> PE transpose back to [px, co] per 2-row tile
      -> VectorE fused relu + PSUM evacuation into a per-image staging
         buffer -> 2 large NHWC stores per image via GPSIMD (SWDGE).
  - Host precomputes constants only: sign(kernel) in fp8 DoubleRow layout,
    folded BN scale/shift, and a 1-ulp nudge of x where fl(x+bias1)==0 so
    device Sign (sign(0)=0) matches ste_sign (sign(0)=+1).
"""

import numpy as np
import ml_dtypes

import concourse.bass as bass
import concourse.mybir as mybir
import concourse.tile as tile
from concourse import bacc
from concourse.bass_utils import run_bass_kernel_spmd
from concourse.masks import make_identity
from concourse.tile_rust import add_dep_helper

# Problem shape (hardcoded per contract).
B, H, W, CIN, COUT = 32, 56, 56, 256, 256
N_CORES = 8
IMG = B // N_CORES          # images per core
EPS = 1e-3

P = 128
WPAD = 58                   # padded row width (56 + 2)
ROWS = 59                   # 1 top pad + 56 real + 1 bottom pad + slack
AFREE = 3424                # ROWS*WPAD=3422 padded to %16 for DoubleRow APs
RP = H // 2                 # 28 row-pairs per image
G8 = H // 8                 # 7 eight-row groups per image
NPX = 8 * WPAD              # 464 psum pixels per conv group

F32 = mybir.dt.float32
FP8 = mybir.dt.float8e4

AluOp = mybir.AluOpType


def _build_program():
    nc = bacc.Bacc(
        "TRN2",
        target_bir_lowering=False,
        debug=False,
        enable_asserts=False,
        num_devices=N_CORES,
    )

    x_ap = nc.dram_tensor("x", (IMG, H, W, CIN), F32, kind="ExternalInput").ap()
    w_ap = nc.dram_tensor("wq", (P, 9, 2, 2 * P), FP8, kind="ExternalInput").ap()
    b1_ap = nc.dram_tensor("b1", (2, P), F32, kind="ExternalInput").ap()
    s_ap = nc.dram_tensor("s", (2, P), F32, kind="ExternalInput").ap()
    t_ap = nc.dram_tensor("t", (2, P), F32, kind="ExternalInput").ap()
    out_ap = nc.dram_tensor("out", (IMG, H, W, COUT), F32, kind="ExternalOutput").ap()

    x_flat = x_ap.rearrange("b h w c -> b (h w) c")

    with tile.TileContext(nc) as tc:
        with (
            tc.tile_pool(name="const", bufs=1) as const_pool,
            tc.tile_pool(name="xin", bufs=4) as x_pool,
            tc.tile_pool(name="ybn", bufs=16) as y_pool,
            tc.tile_pool(name="pst", bufs=2, space="PSUM") as pst_pool,
            tc.tile_pool(name="pso", bufs=4, space="PSUM") as pso_pool,
            tc.tile_pool(name="psu", bufs=2, space="PSUM") as psu_pool,
        ):
            w_sb = const_pool.tile([P, 9, 2, 2 * P], FP8)
            nc.sync.dma_start(w_sb[:], w_ap)
            b1_sb = const_pool.tile([P, 2], F32)
            nc.sync.dma_start(b1_sb[:], b1_ap.rearrange("t p -> p t"))
            s_sb = const_pool.tile([P, 2], F32)
            nc.sync.dma_start(s_sb[:], s_ap.rearrange("t p -> p t"))
            t_sb = const_pool.tile([P, 2], F32)
            nc.sync.dma_start(t_sb[:], t_ap.rearrange("t p -> p t"))
            ident = const_pool.tile([P, P], F32)
            make_identity(nc, ident[:])

            # Double-buffered fp8 padded activation buffers + f32 output stage.
            acts = [
                const_pool.tile([P, 2, AFREE], FP8, name=f"act{i}") for i in range(2)
            ]
            ubig = [
                const_pool.tile([P, RP, COUT], F32, name=f"ubig{i}") for i in range(2)
            ]
            nc.gpsimd.memset(acts[0][:], 0.0)
            nc.gpsimd.memset(acts[1][:], 0.0)

            for b in range(IMG):
                slot = b % 2
                act = acts[slot]
                ub = ubig[slot]

                # ---- load + transpose + binarize (8-row groups) ----
                for g in range(G8):
                    xt = x_pool.tile([112, 4, CIN], F32)
                    nc.sync.dma_start(
                        xt[:],
                        x_flat[b, 448 * g : 448 * (g + 1), :].rearrange(
                            "(p j) c -> p j c", p=112
                        ),
                    )
                    for j in range(4):
                        for ci in range(2):
                            pt = pst_pool.tile([P, 112], F32)
                            nc.tensor.transpose(
                                pt[:],
                                xt[:, j, ci * P : (ci + 1) * P],
                                ident[:112, :112],
                            )
                            # px = 4p + j -> row r = px//56 (8 rows), col 4q+j.
                            # dest: padded rows 8g+1..8g+8, cols 2+j step 4.
                            base = (8 * g + 1) * WPAD + 2 + j
                            dest = (
                                act[:, ci, base : base + 8 * WPAD]
                                .rearrange("p (r w) -> p r w", w=WPAD)[:, :, 0:56]
                                .rearrange("p r (q x) -> p r q x", x=4)[:, :, :, 0]
                            )
                            nc.scalar.activation(
                                dest,
                                pt.rearrange("p (r q) -> p r q", q=14),
                                mybir.ActivationFunctionType.Sign,
                                bias=b1_sb[:, ci : ci + 1],
                                scale=1.0,
                            )

                # ---- conv (fp8 DoubleRow, weights stationary) + BN affine ----
                y_tiles = {}
                for co in range(2):
                    for block in ((0, 1, 2, 3), (4, 5, 6)):
                        pm = {
                            m: pso_pool.tile([P, NPX], F32, name="pm", tag="pm")
                            for m in block
                        }
                        for tap in range(9):
                            dh, dw = tap // 3, tap % 3
                            for m in block:
                                rbase = (8 * m + dh) * WPAD + dw
                                nc.tensor.matmul(
                                    pm[m][:],
                                    w_sb[:, tap, co],
                                    act[:, :, rbase : rbase + NPX],
                                    start=(tap == 0),
                                    stop=(tap == 8),
                                    perf_mode=mybir.MatmulPerfMode.DoubleRowSwInterleave,
                                )
                        for m in block:
                            y = y_pool.tile([P, NPX], F32, name="y", tag="y")
                            # y = conv * scale + shift   (per-partition co consts)
                            nc.vector.tensor_scalar(
                                y[:], pm[m][:],
                                s_sb[:, co : co + 1], t_sb[:, co : co + 1],
                                op0=AluOp.mult, op1=AluOp.add,
                            )
                            y_tiles[(co, m)] = y

                # ---- transpose back to [px, co], fused relu, stage, store ----
                for m in range(G8):
                    for r in range(4):
                        k = 4 * m + r
                        pu = psu_pool.tile([116, COUT], F32)
                        for co in range(2):
                            nc.tensor.matmul(
                                pu[:, co * P : (co + 1) * P],
                                y_tiles[(co, m)][:, 116 * r : 116 * r + 116],
                                ident[:, :P],
                                is_transpose=True,
                                start=(co == 0),
                                stop=(co == 1),
                            )
                        # partitions: 0 pad | 1..56 row 2k | 57,58 pad |
                        # 59..114 row 2k+1 | 115 pad
                        nc.vector.tensor_scalar(
                            ub[:116, k, :], pu[:], 0.0, None, op0=AluOp.max
                        )

                ev = out_ap[b].rearrange("(k two) w c -> w two k c", two=2)
                nc.gpsimd.dma_start(ev[:, 0], ub[1:57])
                nc.gpsimd.dma_start(ev[:, 1], ub[59:115])

    nc.compile()
    return nc


_NC_CACHE = None


def _get_nc():
    global _NC_CACHE
    if _NC_CACHE is None:
        _NC_CACHE = _build_program()
    return _NC_CACHE


def _prep_inputs(x, bias1, kernel, bn_beta, bn_mean, bn_var, bias2):
    x = np.asarray(x, dtype=np.float32)
    bias1 = np.asarray(bias1, dtype=np.float32)
    kernel = np.asarray(kernel, dtype=np.float32)
    bn_beta = np.asarray(bn_beta, dtype=np.float32)
    bn_mean = np.asarray(bn_mean, dtype=np.float32)
    bn_var = np.asarray(bn_var, dtype=np.float32)
    bias2 = np.asarray(bias2, dtype=np.float32).reshape(-1)

    # Device computes sign(fl(x + b)) with sign(0)=0; the reference wants
    # sign(0)=+1. Nudge x by 1 ulp wherever fl(x+b) == 0 exactly (x is only
    # consumed through this sign).
    z = x + bias1
    if np.any(z == 0.0):
        x = np.where(z == 0.0, np.nextafter(x, np.float32(np.inf)), x)

    # Weights: ste_sign with sign(0)=+1, exact in fp8e4.
    # DoubleRowSwInterleave stationary layout per (tap, co_t):
    # [A127 B127 A126 B126 ... A0 B0] where A/B are the ci halves
    # (ci = o*128 + ki, matching the act buffer's [ki, ci_t, px] pairing)
    # and columns are stored co-reversed.
    wq = np.where(kernel >= 0, np.float32(1.0), np.float32(-1.0))
    wq = wq.reshape(9, 2, P, 2, P)[..., ::-1]       # [tap, o, ki, co_t, m]
    wq = wq.transpose(2, 0, 3, 4, 1)                # [ki, tap, co_t, m, o]
    wq = np.ascontiguousarray(wq).reshape(P, 9, 2, 2 * P)
    wq = wq.astype(ml_dtypes.float8_e4m3)

    s = (1.0 / np.sqrt(bn_var + np.float32(EPS))).astype(np.float32)
    t = (bn_beta - bn_mean * s + bias2).astype(np.float32)
    b1 = np.ascontiguousarray(bias1.reshape(2, P)).astype(np.float32)

    in_maps = []
    for c in range(N_CORES):
        in_maps.append(
            {
                "x": np.ascontiguousarray(x[c * IMG : (c + 1) * IMG]),
                "wq": wq,
                "b1": b1,
                "s": np.ascontiguousarray(s.reshape(2, P)),
                "t": np.ascontiguousarray(t.reshape(2, P)),
            }
        )
    return in_maps


def _ensure_ntff_hook():
    """This container ships the NTFF profiling machinery but not the
    ``antenv.axon_hooks`` shim module bass_utils imports it through;
    synthesize it so trace=True can capture HW exec times."""
    import sys
    import types

    if "antenv.axon_hooks" in sys.modules:
        return
    import antenv
    from trn_agent_boot.trn_boot import _ntff_profile_via_ctypes

    hook = _ntff_profile_via_ctypes("/opt/axon/libaxon_pjrt.so")
    mod = types.ModuleType("antenv.axon_hooks")
    mod.get_axon_ntff_profile_hook = lambda: hook
    mod.set_axon_ntff_profile_hook = lambda h: None
    sys.modules["antenv.axon_hooks"] = mod
    antenv.axon_hooks = mod


def run(inputs: dict, trace: bool = False):
    """Run the SPMD kernel. Returns (out, exec_time_ns or None)."""
    nc = _get_nc()
    in_maps = _prep_inputs(**inputs)
    if trace:
        try:
            _ensure_ntff_hook()
        except Exception as e:  # degrade to untraced run
            print(f"ntff hook unavailable: {e}")
    res = run_bass_kernel_spmd(
        nc, in_maps, core_ids=list(range(N_CORES)), trace=trace
    )
    out = np.concatenate([r["out"] for r in res.results], axis=0)
    return out, res.exec_time_ns


def kernel(**inputs) -> np.ndarray:
    out, _ = run(inputs, trace=False)
    return out


# revision 24
# speedup vs baseline: 1.4248x; 1.0535x over previous
"""Trainium2 Bass kernel for nn_BasicBlock (binarized 3x3 conv + BN + ReLU).

Reference computation (NHWC, f32):
    a   = ste_sign(x + bias1)            # +-1, sign(0)=+1
    qk  = ste_sign(kernel)               # +-1
    y   = conv2d(a, qk, SAME, stride 1)  # (32,56,56,256)
    y   = (y - mean) * rsqrt(var+eps) + beta
    out = relu(y + bias2)

Strategy (v2, fp8 DoubleRow):
  - Data-parallel over batch: 8 cores x 4 images, no collectives.
  - Operands are exactly +-1, exact in fp8e4; fp32 PSUM accumulation keeps
    integer conv sums (|y| <= 2304) bit-exact. DoubleRow packs both
    128-channel Cin halves into one matmul at 2 MACs/cell/cycle.
  - Per core pipeline, per image:
      load 8-row groups [112 part, 4 px, 256ch] (4KB/partition descriptors)
      -> PE transpose f32 -> ScalarE Sign(x+bias1) -> fp8 +-1 into a
      zero-padded 58-wide channel-major image buffer
      -> conv: weights-stationary fp8 DoubleRow matmuls, psum [co,464px]
         (8 output rows per group, 9 taps accumulated, 2 Cout tiles)
      -> VectorE BN affine (y*s + t, per-partition scale/shift)
      -> PE transpose back to [px, co] per 2-row tile
      -> VectorE fused relu + PSUM evacuation into a per-image staging
         buffer -> 2 large NHWC stores per image via GPSIMD (SWDGE).
  - Host precomputes constants only: sign(kernel) in fp8 DoubleRow layout,
    folded BN scale/shift, and a 1-ulp nudge of x where fl(x+bias1)==0 so
    device Sign (sign(0)=0) matches ste_sign (sign(0)=+1).
"""

import numpy as np
import ml_dtypes

import concourse.bass as bass
import concourse.mybir as mybir
import concourse.tile as tile
from concourse import bacc
from concourse.bass_utils import run_bass_kernel_spmd
from concourse.masks import make_identity
from concourse.tile_rust import add_dep_helper

# Problem shape (hardcoded per contract).
B, H, W, CIN, COUT = 32, 56, 56, 256, 256
N_CORES = 8
IMG = B // N_CORES          # images per core
EPS = 1e-3

P = 128
WPAD = 58                   # padded row width (56 + 2)
ROWS = 59                   # 1 top pad + 56 real + 1 bottom pad + slack
AFREE = 3424                # ROWS*WPAD=3422 padded to %16 for DoubleRow APs
RP = H // 2                 # 28 row-pairs per image
G8 = H // 8                 # 7 eight-row groups per image
NPX = 8 * WPAD              # 464 psum pixels per conv group

F32 = mybir.dt.float32
FP8 = mybir.dt.float8e4

AluOp = mybir.AluOpType


def _build_program():
    nc = bacc.Bacc(
        "TRN2",
        target_bir_lowering=False,
        debug=False,
        enable_asserts=False,
        num_devices=N_CORES,
    )

    x_ap = nc.dram_tensor("x", (IMG, H, W, CIN), F32, kind="ExternalInput").ap()
    w_ap = nc.dram_tensor("wq", (P, 9, 2, 2 * P), FP8, kind="ExternalInput").ap()
    b1_ap = nc.dram_tensor("b1", (2, P), F32, kind="ExternalInput").ap()
    s_ap = nc.dram_tensor("s", (2, P), F32, kind="ExternalInput").ap()
    t_ap = nc.dram_tensor("t", (2, P), F32, kind="ExternalInput").ap()
    out_ap = nc.dram_tensor("out", (IMG, H, W, COUT), F32, kind="ExternalOutput").ap()

    x_flat = x_ap.rearrange("b h w c -> b (h w) c")

    with tile.TileContext(nc) as tc:
        with (
            tc.tile_pool(name="const", bufs=1) as const_pool,
            tc.tile_pool(name="xin", bufs=4) as x_pool,
            tc.tile_pool(name="ybn", bufs=16) as y_pool,
            tc.tile_pool(name="pst", bufs=3, space="PSUM") as pst_pool,
            tc.tile_pool(name="pso", bufs=3, space="PSUM") as pso_pool,
            tc.tile_pool(name="psu", bufs=2, space="PSUM") as psu_pool,
        ):
            # Identity + activation-pad memsets first so the first image's
            # transposes aren't queued behind the large constant DMAs.
            ident = const_pool.tile([P, P], F32)
            make_identity(nc, ident[:])
            acts = [
                const_pool.tile([P, 2, AFREE], FP8, name=f"act{i}") for i in range(2)
            ]
            nc.vector.memset(acts[0][:], 0.0)
            nc.vector.memset(acts[1][:], 0.0)

            b1_sb = const_pool.tile([P, 2], F32)
            nc.sync.dma_start(b1_sb[:], b1_ap.rearrange("t p -> p t"))
            w_sb = const_pool.tile([P, 9, 2, 2 * P], FP8)
            nc.sync.dma_start(w_sb[:], w_ap)
            s_sb = const_pool.tile([P, 2], F32)
            nc.sync.dma_start(s_sb[:], s_ap.rearrange("t p -> p t"))
            t_sb = const_pool.tile([P, 2], F32)
            nc.sync.dma_start(t_sb[:], t_ap.rearrange("t p -> p t"))

            ubig = [
                const_pool.tile([P, RP, COUT], F32, name=f"ubig{i}") for i in range(2)
            ]

            for b in range(IMG):
                slot = b % 2
                act = acts[slot]
                ub = ubig[slot]

                # ---- load + transpose + binarize (8-row groups) ----
                for g in range(G8):
                    xt = x_pool.tile([112, 4, CIN], F32)
                    nc.sync.dma_start(
                        xt[:],
                        x_flat[b, 448 * g : 448 * (g + 1), :].rearrange(
                            "(p j) c -> p j c", p=112
                        ),
                    )
                    for j in range(4):
                        for ci in range(2):
                            pt = pst_pool.tile([P, 112], F32)
                            nc.tensor.transpose(
                                pt[:],
                                xt[:, j, ci * P : (ci + 1) * P],
                                ident[:112, :112],
                            )
                            # px = 4p + j -> row r = px//56 (8 rows), col 4q+j.
                            # dest: padded rows 8g+1..8g+8, cols 2+j step 4.
                            base = (8 * g + 1) * WPAD + 2 + j
                            dest = (
                                act[:, ci, base : base + 8 * WPAD]
                                .rearrange("p (r w) -> p r w", w=WPAD)[:, :, 0:56]
                                .rearrange("p r (q x) -> p r q x", x=4)[:, :, :, 0]
                            )
                            nc.scalar.activation(
                                dest,
                                pt.rearrange("p (r q) -> p r q", q=14),
                                mybir.ActivationFunctionType.Sign,
                                bias=b1_sb[:, ci : ci + 1],
                                scale=1.0,
                            )

                # ---- conv (fp8 SwInterleave, weights stationary) + BN affine ----
                y_tiles = {}
                for co in range(2):
                    for m in range(G8):
                        pm = pso_pool.tile([P, NPX], F32, name="pm", tag="pm")
                        for tap in range(9):
                            dh, dw = tap // 3, tap % 3
                            rbase = (8 * m + dh) * WPAD + dw
                            nc.tensor.matmul(
                                pm[:],
                                w_sb[:, tap, co],
                                act[:, :, rbase : rbase + NPX],
                                start=(tap == 0),
                                stop=(tap == 8),
                                perf_mode=mybir.MatmulPerfMode.DoubleRowSwInterleave,
                            )
                        y = y_pool.tile([P, NPX], F32, name="y", tag="y")
                        # y = conv * scale + shift   (per-partition co consts)
                        nc.vector.tensor_scalar(
                            y[:], pm[:],
                            s_sb[:, co : co + 1], t_sb[:, co : co + 1],
                            op0=AluOp.mult, op1=AluOp.add,
                        )
                        y_tiles[(co, m)] = y

                # ---- transpose back to [px, co], fused relu, stage, store ----
                ev = out_ap[b].rearrange("(k two) w c -> w two k c", two=2)
                for m in range(G8):
                    for r in range(4):
                        k = 4 * m + r
                        pu = psu_pool.tile([116, COUT], F32)
                        for co in range(2):
                            nc.tensor.matmul(
                                pu[:, co * P : (co + 1) * P],
                                y_tiles[(co, m)][:, 116 * r : 116 * r + 116],
                                ident[:, :P],
                                is_transpose=True,
                                start=(co == 0),
                                stop=(co == 1),
                            )
                        # partitions: 0 pad | 1..56 row 2k | 57,58 pad |
                        # 59..114 row 2k+1 | 115 pad
                        nc.vector.tensor_scalar(
                            ub[:116, k, :], pu[:], 0.0, None, op0=AluOp.max
                        )
                    # store the 8 finished rows (4 row-pairs) of this group,
                    # split across the two DGE paths so issue overlaps.
                    ksl = slice(4 * m, 4 * m + 4)
                    nc.sync.dma_start(ev[:, 0, ksl], ub[1:57, ksl])
                    nc.gpsimd.dma_start(ev[:, 1, ksl], ub[59:115, ksl])

    nc.compile()
    return nc


_NC_CACHE = None


def _get_nc():
    global _NC_CACHE
    if _NC_CACHE is None:
        _NC_CACHE = _build_program()
    return _NC_CACHE


def _prep_inputs(x, bias1, kernel, bn_beta, bn_mean, bn_var, bias2):
    x = np.asarray(x, dtype=np.float32)
    bias1 = np.asarray(bias1, dtype=np.float32)
    kernel = np.asarray(kernel, dtype=np.float32)
    bn_beta = np.asarray(bn_beta, dtype=np.float32)
    bn_mean = np.asarray(bn_mean, dtype=np.float32)
    bn_var = np.asarray(bn_var, dtype=np.float32)
    bias2 = np.asarray(bias2, dtype=np.float32).reshape(-1)

    # Device computes sign(fl(x + b)) with sign(0)=0; the reference wants
    # sign(0)=+1. Nudge x by 1 ulp wherever fl(x+b) == 0 exactly (x is only
    # consumed through this sign).
    z = x + bias1
    if np.any(z == 0.0):
        x = np.where(z == 0.0, np.nextafter(x, np.float32(np.inf)), x)

    # Weights: ste_sign with sign(0)=+1, exact in fp8e4.
    # DoubleRowSwInterleave stationary layout per (tap, co_t):
    # [A127 B127 A126 B126 ... A0 B0] where A/B are the ci halves
    # (ci = o*128 + ki, matching the act buffer's [ki, ci_t, px] pairing)
    # and columns are stored co-reversed.
    wq = np.where(kernel >= 0, np.float32(1.0), np.float32(-1.0))
    wq = wq.reshape(9, 2, P, 2, P)[..., ::-1]       # [tap, o, ki, co_t, m]
    wq = wq.transpose(2, 0, 3, 4, 1)                # [ki, tap, co_t, m, o]
    wq = np.ascontiguousarray(wq).reshape(P, 9, 2, 2 * P)
    wq = wq.astype(ml_dtypes.float8_e4m3)

    s = (1.0 / np.sqrt(bn_var + np.float32(EPS))).astype(np.float32)
    t = (bn_beta - bn_mean * s + bias2).astype(np.float32)
    b1 = np.ascontiguousarray(bias1.reshape(2, P)).astype(np.float32)

    in_maps = []
    for c in range(N_CORES):
        in_maps.append(
            {
                "x": np.ascontiguousarray(x[c * IMG : (c + 1) * IMG]),
                "wq": wq,
                "b1": b1,
                "s": np.ascontiguousarray(s.reshape(2, P)),
                "t": np.ascontiguousarray(t.reshape(2, P)),
            }
        )
    return in_maps


def _ensure_ntff_hook():
    """This container ships the NTFF profiling machinery but not the
    ``antenv.axon_hooks`` shim module bass_utils imports it through;
    synthesize it so trace=True can capture HW exec times."""
    import sys
    import types

    if "antenv.axon_hooks" in sys.modules:
        return
    import antenv
    from trn_agent_boot.trn_boot import _ntff_profile_via_ctypes

    hook = _ntff_profile_via_ctypes("/opt/axon/libaxon_pjrt.so")
    mod = types.ModuleType("antenv.axon_hooks")
    mod.get_axon_ntff_profile_hook = lambda: hook
    mod.set_axon_ntff_profile_hook = lambda h: None
    sys.modules["antenv.axon_hooks"] = mod
    antenv.axon_hooks = mod


def run(inputs: dict, trace: bool = False):
    """Run the SPMD kernel. Returns (out, exec_time_ns or None)."""
    nc = _get_nc()
    in_maps = _prep_inputs(**inputs)
    if trace:
        try:
            _ensure_ntff_hook()
        except Exception as e:  # degrade to untraced run
            print(f"ntff hook unavailable: {e}")
    res = run_bass_kernel_spmd(
        nc, in_maps, core_ids=list(range(N_CORES)), trace=trace
    )
    out = np.concatenate([r["out"] for r in res.results], axis=0)
    return out, res.exec_time_ns


def kernel(**inputs) -> np.ndarray:
    out, _ = run(inputs, trace=False)
    return out


# revision 27
# speedup vs baseline: 1.5546x; 1.0911x over previous
"""Trainium2 Bass kernel for nn_BasicBlock (binarized 3x3 conv + BN + ReLU).

Reference computation (NHWC, f32):
    a   = ste_sign(x + bias1)            # +-1, sign(0)=+1
    qk  = ste_sign(kernel)               # +-1
    y   = conv2d(a, qk, SAME, stride 1)  # (32,56,56,256)
    y   = (y - mean) * rsqrt(var+eps) + beta
    out = relu(y + bias2)

Strategy (v2, fp8 DoubleRow):
  - Data-parallel over batch: 8 cores x 4 images, no collectives.
  - Operands are exactly +-1, exact in fp8e4; fp32 PSUM accumulation keeps
    integer conv sums (|y| <= 2304) bit-exact. DoubleRow packs both
    128-channel Cin halves into one matmul at 2 MACs/cell/cycle.
  - Per core pipeline, per image:
      load 8-row groups [112 part, 4 px, 256ch] (4KB/partition descriptors)
      -> PE transpose f32 -> ScalarE Sign(x+bias1) -> fp8 +-1 into a
      zero-padded 58-wide channel-major image buffer
      -> conv: weights-stationary fp8 DoubleRow matmuls, psum [co,464px]
         (8 output rows per group, 9 taps accumulated, 2 Cout tiles)
      -> VectorE BN affine (y*s + t, per-partition scale/shift)
      -> PE transpose back to [px, co] per 2-row tile
      -> VectorE fused relu + PSUM evacuation into a per-image staging
         buffer -> 2 large NHWC stores per image via GPSIMD (SWDGE).
  - Host precomputes constants only: sign(kernel) in fp8 DoubleRow layout,
    folded BN scale/shift, and a 1-ulp nudge of x where fl(x+bias1)==0 so
    device Sign (sign(0)=0) matches ste_sign (sign(0)=+1).
"""

import numpy as np
import ml_dtypes

import concourse.bass as bass
import concourse.mybir as mybir
import concourse.tile as tile
from concourse import bacc
from concourse.bass_utils import run_bass_kernel_spmd
from concourse.masks import make_identity
from concourse.tile_rust import add_dep_helper

# Problem shape (hardcoded per contract).
B, H, W, CIN, COUT = 32, 56, 56, 256, 256
N_CORES = 8
IMG = B // N_CORES          # images per core
EPS = 1e-3

P = 128
WPAD = 58                   # padded row width (56 + 2)
ROWS = 59                   # 1 top pad + 56 real + 1 bottom pad + slack
AFREE = 3424                # ROWS*WPAD=3422 padded to %16 for DoubleRow APs
RP = H // 2                 # 28 row-pairs per image
G8 = H // 8                 # 7 eight-row groups per image
NPX = 8 * WPAD              # 464 psum pixels per conv group

F32 = mybir.dt.float32
FP8 = mybir.dt.float8e4

AluOp = mybir.AluOpType


def _build_program():
    nc = bacc.Bacc(
        "TRN2",
        target_bir_lowering=False,
        debug=False,
        enable_asserts=False,
        num_devices=N_CORES,
    )

    x_ap = nc.dram_tensor("x", (IMG, H, W, CIN), F32, kind="ExternalInput").ap()
    w_ap = nc.dram_tensor("wq", (P, 9, 2, 2 * P), FP8, kind="ExternalInput").ap()
    b1_ap = nc.dram_tensor("b1", (2, P), F32, kind="ExternalInput").ap()
    s_ap = nc.dram_tensor("s", (2, P), F32, kind="ExternalInput").ap()
    t_ap = nc.dram_tensor("t", (2, P), F32, kind="ExternalInput").ap()
    out_ap = nc.dram_tensor("out", (IMG, H, W, COUT), F32, kind="ExternalOutput").ap()

    x_flat = x_ap.rearrange("b h w c -> b (h w) c")

    with tile.TileContext(nc) as tc:
        with (
            tc.tile_pool(name="const", bufs=1) as const_pool,
            tc.tile_pool(name="xin", bufs=4) as x_pool,
            tc.tile_pool(name="ybn", bufs=16) as y_pool,
            tc.tile_pool(name="pst", bufs=3, space="PSUM") as pst_pool,
            tc.tile_pool(name="pso", bufs=3, space="PSUM") as pso_pool,
            tc.tile_pool(name="psu", bufs=2, space="PSUM") as psu_pool,
        ):
            # Identity + activation-pad memsets first so the first image's
            # transposes/signs aren't queued behind the large constant DMAs.
            ident = const_pool.tile([P, P], F32)
            make_identity(nc, ident[:])
            acts = [
                const_pool.tile([P, 2, AFREE], FP8, name=f"act{i}") for i in range(2)
            ]
            nc.gpsimd.memset(acts[0][:], 0.0)
            nc.gpsimd.memset(acts[1][:], 0.0)

            b1_sb = const_pool.tile([P, 2], F32)
            nc.sync.dma_start(b1_sb[:], b1_ap.rearrange("t p -> p t"))
            # w/s/t are first needed by image 0's conv phase; issued inside
            # the b==0 iteration, after its input loads.
            w_sb = const_pool.tile([P, 9, 2, 2 * P], FP8)
            s_sb = const_pool.tile([P, 2], F32)
            t_sb = const_pool.tile([P, 2], F32)

            ubig = [
                const_pool.tile([P, RP, COUT], F32, name=f"ubig{i}") for i in range(2)
            ]

            for b in range(IMG):
                slot = b % 2
                act = acts[slot]
                ub = ubig[slot]

                # ---- load + transpose + binarize (8-row groups) ----
                for g in range(G8):
                    xt = x_pool.tile([112, 4, CIN], F32)
                    nc.sync.dma_start(
                        xt[:],
                        x_flat[b, 448 * g : 448 * (g + 1), :].rearrange(
                            "(p j) c -> p j c", p=112
                        ),
                    )
                    for j in range(4):
                        for ci in range(2):
                            pt = pst_pool.tile([P, 112], F32)
                            nc.tensor.transpose(
                                pt[:],
                                xt[:, j, ci * P : (ci + 1) * P],
                                ident[:112, :112],
                            )
                            # px = 4p + j -> row r = px//56 (8 rows), col 4q+j.
                            # dest: padded rows 8g+1..8g+8, cols 2+j step 4.
                            base = (8 * g + 1) * WPAD + 2 + j
                            dest = (
                                act[:, ci, base : base + 8 * WPAD]
                                .rearrange("p (r w) -> p r w", w=WPAD)[:, :, 0:56]
                                .rearrange("p r (q x) -> p r q x", x=4)[:, :, :, 0]
                            )
                            nc.scalar.activation(
                                dest,
                                pt.rearrange("p (r q) -> p r q", q=14),
                                mybir.ActivationFunctionType.Sign,
                                bias=b1_sb[:, ci : ci + 1],
                                scale=1.0,
                            )

                if b == 0:
                    nc.sync.dma_start(w_sb[:], w_ap)
                    nc.sync.dma_start(s_sb[:], s_ap.rearrange("t p -> p t"))
                    nc.sync.dma_start(t_sb[:], t_ap.rearrange("t p -> p t"))

                # ---- conv (fp8 SwInterleave, weights stationary) + BN affine ----
                y_tiles = {}
                for co in range(2):
                    for m in range(G8):
                        pm = pso_pool.tile([P, NPX], F32, name="pm", tag="pm")
                        for tap in range(9):
                            dh, dw = tap // 3, tap % 3
                            rbase = (8 * m + dh) * WPAD + dw
                            nc.tensor.matmul(
                                pm[:],
                                w_sb[:, tap, co],
                                act[:, :, rbase : rbase + NPX],
                                start=(tap == 0),
                                stop=(tap == 8),
                                perf_mode=mybir.MatmulPerfMode.DoubleRowSwInterleave,
                            )
                        y = y_pool.tile([P, NPX], F32, name="y", tag="y")
                        # y = conv * scale + shift   (per-partition co consts)
                        nc.vector.tensor_scalar(
                            y[:], pm[:],
                            s_sb[:, co : co + 1], t_sb[:, co : co + 1],
                            op0=AluOp.mult, op1=AluOp.add,
                        )
                        y_tiles[(co, m)] = y

                # ---- transpose back to [px, co], fused relu, stage, store ----
                ev = out_ap[b].rearrange("(k two) w c -> w two k c", two=2)
                for m in range(G8):
                    for r in range(4):
                        k = 4 * m + r
                        pu = psu_pool.tile([116, COUT], F32)
                        for co in range(2):
                            nc.tensor.matmul(
                                pu[:, co * P : (co + 1) * P],
                                y_tiles[(co, m)][:, 116 * r : 116 * r + 116],
                                ident[:, :P],
                                is_transpose=True,
                                start=(co == 0),
                                stop=(co == 1),
                            )
                        # partitions: 0 pad | 1..56 row 2k | 57,58 pad |
                        # 59..114 row 2k+1 | 115 pad
                        nc.vector.tensor_scalar(
                            ub[:116, k, :], pu[:], 0.0, None, op0=AluOp.max
                        )
                    # store the 8 finished rows (4 row-pairs) of this group,
                    # split across the two DGE paths so issue overlaps.
                    ksl = slice(4 * m, 4 * m + 4)
                    nc.gpsimd.dma_start(ev[:, 0, ksl], ub[1:57, ksl])
                    nc.gpsimd.dma_start(ev[:, 1, ksl], ub[59:115, ksl])

    nc.compile()
    return nc


_NC_CACHE = None


def _get_nc():
    global _NC_CACHE
    if _NC_CACHE is None:
        _NC_CACHE = _build_program()
    return _NC_CACHE


def _prep_inputs(x, bias1, kernel, bn_beta, bn_mean, bn_var, bias2):
    x = np.asarray(x, dtype=np.float32)
    bias1 = np.asarray(bias1, dtype=np.float32)
    kernel = np.asarray(kernel, dtype=np.float32)
    bn_beta = np.asarray(bn_beta, dtype=np.float32)
    bn_mean = np.asarray(bn_mean, dtype=np.float32)
    bn_var = np.asarray(bn_var, dtype=np.float32)
    bias2 = np.asarray(bias2, dtype=np.float32).reshape(-1)

    # Device computes sign(fl(x + b)) with sign(0)=0; the reference wants
    # sign(0)=+1. Nudge x by 1 ulp wherever fl(x+b) == 0 exactly (x is only
    # consumed through this sign).
    z = x + bias1
    if np.any(z == 0.0):
        x = np.where(z == 0.0, np.nextafter(x, np.float32(np.inf)), x)

    # Weights: ste_sign with sign(0)=+1, exact in fp8e4.
    # DoubleRowSwInterleave stationary layout per (tap, co_t):
    # [A127 B127 A126 B126 ... A0 B0] where A/B are the ci halves
    # (ci = o*128 + ki, matching the act buffer's [ki, ci_t, px] pairing)
    # and columns are stored co-reversed.
    wq = np.where(kernel >= 0, np.float32(1.0), np.float32(-1.0))
    wq = wq.reshape(9, 2, P, 2, P)[..., ::-1]       # [tap, o, ki, co_t, m]
    wq = wq.transpose(2, 0, 3, 4, 1)                # [ki, tap, co_t, m, o]
    wq = np.ascontiguousarray(wq).reshape(P, 9, 2, 2 * P)
    wq = wq.astype(ml_dtypes.float8_e4m3)

    s = (1.0 / np.sqrt(bn_var + np.float32(EPS))).astype(np.float32)
    t = (bn_beta - bn_mean * s + bias2).astype(np.float32)
    b1 = np.ascontiguousarray(bias1.reshape(2, P)).astype(np.float32)

    in_maps = []
    for c in range(N_CORES):
        in_maps.append(
            {
                "x": np.ascontiguousarray(x[c * IMG : (c + 1) * IMG]),
                "wq": wq,
                "b1": b1,
                "s": np.ascontiguousarray(s.reshape(2, P)),
                "t": np.ascontiguousarray(t.reshape(2, P)),
            }
        )
    return in_maps


def _ensure_ntff_hook():
    """This container ships the NTFF profiling machinery but not the
    ``antenv.axon_hooks`` shim module bass_utils imports it through;
    synthesize it so trace=True can capture HW exec times."""
    import sys
    import types

    if "antenv.axon_hooks" in sys.modules:
        return
    import antenv
    from trn_agent_boot.trn_boot import _ntff_profile_via_ctypes

    hook = _ntff_profile_via_ctypes("/opt/axon/libaxon_pjrt.so")
    mod = types.ModuleType("antenv.axon_hooks")
    mod.get_axon_ntff_profile_hook = lambda: hook
    mod.set_axon_ntff_profile_hook = lambda h: None
    sys.modules["antenv.axon_hooks"] = mod
    antenv.axon_hooks = mod


def run(inputs: dict, trace: bool = False):
    """Run the SPMD kernel. Returns (out, exec_time_ns or None)."""
    nc = _get_nc()
    in_maps = _prep_inputs(**inputs)
    if trace:
        try:
            _ensure_ntff_hook()
        except Exception as e:  # degrade to untraced run
            print(f"ntff hook unavailable: {e}")
    res = run_bass_kernel_spmd(
        nc, in_maps, core_ids=list(range(N_CORES)), trace=trace
    )
    out = np.concatenate([r["out"] for r in res.results], axis=0)
    return out, res.exec_time_ns


def kernel(**inputs) -> np.ndarray:
    out, _ = run(inputs, trace=False)
    return out


# revision 34
# speedup vs baseline: 1.6406x; 1.0553x over previous
"""Trainium2 Bass kernel for nn_BasicBlock (binarized 3x3 conv + BN + ReLU).

Reference computation (NHWC, f32):
    a   = ste_sign(x + bias1)            # +-1, sign(0)=+1
    qk  = ste_sign(kernel)               # +-1
    y   = conv2d(a, qk, SAME, stride 1)  # (32,56,56,256)
    y   = (y - mean) * rsqrt(var+eps) + beta
    out = relu(y + bias2)

Strategy (v2, fp8 DoubleRow):
  - Data-parallel over batch: 8 cores x 4 images, no collectives.
  - Operands are exactly +-1, exact in fp8e4; fp32 PSUM accumulation keeps
    integer conv sums (|y| <= 2304) bit-exact. DoubleRow packs both
    128-channel Cin halves into one matmul at 2 MACs/cell/cycle.
  - Per core pipeline, per image:
      load 8-row groups [112 part, 4 px, 256ch] (4KB/partition descriptors)
      -> PE transpose f32 -> ScalarE Sign(x+bias1) -> fp8 +-1 into a
      zero-padded 58-wide channel-major image buffer
      -> conv: weights-stationary fp8 DoubleRow matmuls, psum [co,464px]
         (8 output rows per group, 9 taps accumulated, 2 Cout tiles)
      -> VectorE BN affine (y*s + t, per-partition scale/shift)
      -> PE transpose back to [px, co] per 2-row tile
      -> VectorE fused relu + PSUM evacuation into a per-image staging
         buffer -> 2 large NHWC stores per image via GPSIMD (SWDGE).
  - Host precomputes constants only: sign(kernel) in fp8 DoubleRow layout,
    folded BN scale/shift, and a 1-ulp nudge of x where fl(x+bias1)==0 so
    device Sign (sign(0)=0) matches ste_sign (sign(0)=+1).
"""

import numpy as np
import ml_dtypes

import concourse.bass as bass
import concourse.mybir as mybir
import concourse.tile as tile
from concourse import bacc
from concourse.bass_utils import run_bass_kernel_spmd
from concourse.masks import make_identity
from concourse.tile_rust import add_dep_helper

# Problem shape (hardcoded per contract).
B, H, W, CIN, COUT = 32, 56, 56, 256, 256
N_CORES = 8
IMG = B // N_CORES          # images per core
EPS = 1e-3

P = 128
WPAD = 58                   # padded row width (56 + 2)
ROWS = 59                   # 1 top pad + 56 real + 1 bottom pad + slack
AFREE = 3424                # ROWS*WPAD=3422 padded to %16 for DoubleRow APs
RP = H // 2                 # 28 row-pairs per image
G8 = H // 8                 # 7 eight-row groups per image
NPX = 8 * WPAD              # 464 psum pixels per conv group

F32 = mybir.dt.float32
FP8 = mybir.dt.float8e4

AluOp = mybir.AluOpType


def _build_program():
    nc = bacc.Bacc(
        "TRN2",
        target_bir_lowering=False,
        debug=False,
        enable_asserts=False,
        num_devices=N_CORES,
    )

    x_ap = nc.dram_tensor("x", (IMG, H, W, CIN), F32, kind="ExternalInput").ap()
    w_ap = nc.dram_tensor("wq", (P, 9, 2, 2 * P), FP8, kind="ExternalInput").ap()
    b1_ap = nc.dram_tensor("b1", (CIN,), F32, kind="ExternalInput").ap()
    s_ap = nc.dram_tensor("s", (2, P), F32, kind="ExternalInput").ap()
    t_ap = nc.dram_tensor("t", (2, P), F32, kind="ExternalInput").ap()
    out_ap = nc.dram_tensor("out", (IMG, H, W, COUT), F32, kind="ExternalOutput").ap()

    x_flat = x_ap.rearrange("b h w c -> b (h w) c")

    with tile.TileContext(nc) as tc:
        with (
            tc.tile_pool(name="const", bufs=1) as const_pool,
            tc.tile_pool(name="xin", bufs=4) as x_pool,
            tc.tile_pool(name="ybn", bufs=16) as y_pool,
            tc.tile_pool(name="pst", bufs=3, space="PSUM") as pst_pool,
            tc.tile_pool(name="pso", bufs=3, space="PSUM") as pso_pool,
            tc.tile_pool(name="psu", bufs=2, space="PSUM") as psu_pool,
        ):
            # Identity + activation-pad memsets first so the first image's
            # transposes/signs aren't queued behind the large constant DMAs.
            ident = const_pool.tile([P, P], F32)
            make_identity(nc, ident[:])
            identb = const_pool.tile([P, P], mybir.dt.bfloat16)
            make_identity(nc, identb[:])
            acts = [
                const_pool.tile([P, 2, AFREE], FP8, name=f"act{i}") for i in range(2)
            ]
            nc.gpsimd.memset(acts[0][:], 0.0)
            nc.gpsimd.memset(acts[1][:], 0.0)

            b1_sb = const_pool.tile([P, CIN], F32)
            nc.sync.dma_start(b1_sb[:], b1_ap[None, :].to_broadcast((P, CIN)))
            # w/s/t are first needed by image 0's conv phase; issued inside
            # the b==0 iteration, after its input loads.
            w_sb = const_pool.tile([P, 9, 2, 2 * P], FP8)
            s_sb = const_pool.tile([P, 2], F32)
            t_sb = const_pool.tile([P, 2], F32)

            ubig = [
                const_pool.tile([P, RP, COUT], F32, name=f"ubig{i}") for i in range(2)
            ]

            for b in range(IMG):
                slot = b % 2
                act = acts[slot]
                ub = ubig[slot]

                # ---- load + binarize + transpose (8-row groups) ----
                # a = (fl(x + bias1) >= 0) - 0.5  in {-0.5, +0.5} fp8, exact
                # (the conv then yields y/2; the x2 is folded into BN scale).
                for g in range(G8):
                    xt = x_pool.tile([112, 4, CIN], F32)
                    nc.sync.dma_start(
                        xt[:],
                        x_flat[b, 448 * g : 448 * (g + 1), :].rearrange(
                            "(p j) c -> p j c", p=112
                        ),
                    )
                    u = x_pool.tile([112, 4, CIN], F32, name="u", tag="u")
                    nc.vector.tensor_tensor(
                        u[:], xt[:],
                        b1_sb[:112, None, :].to_broadcast((112, 4, CIN)),
                        AluOp.add,
                    )
                    a8 = x_pool.tile([112, 4, CIN], mybir.dt.bfloat16, name="a8", tag="a8")
                    nc.vector.tensor_scalar(
                        a8[:], u[:], 0.0, 0.5, op0=AluOp.is_ge, op1=AluOp.subtract
                    )
                    for j in range(4):
                        for ci in range(2):
                            pt = pst_pool.tile([P, 112], mybir.dt.bfloat16)
                            nc.tensor.transpose(
                                pt[:],
                                a8[:, j, ci * P : (ci + 1) * P],
                                identb[:112, :112],
                            )
                            # px = 4p + j -> row r = px//56 (8 rows), col 4q+j.
                            # dest: padded rows 8g+1..8g+8, cols 2+j step 4.
                            base = (8 * g + 1) * WPAD + 2 + j
                            dest = (
                                act[:, ci, base : base + 8 * WPAD]
                                .rearrange("p (r w) -> p r w", w=WPAD)[:, :, 0:56]
                                .rearrange("p r (q x) -> p r q x", x=4)[:, :, :, 0]
                            )
                            nc.scalar.copy(
                                dest, pt.rearrange("p (r q) -> p r q", q=14)
                            )

                if b == 0:
                    nc.sync.dma_start(w_sb[:], w_ap)
                    nc.sync.dma_start(s_sb[:], s_ap.rearrange("t p -> p t"))
                    nc.sync.dma_start(t_sb[:], t_ap.rearrange("t p -> p t"))

                # ---- conv (fp8 SwInterleave, weights stationary) + BN affine ----
                y_tiles = {}
                for co in range(2):
                    for m in range(G8):
                        pm = pso_pool.tile([P, NPX], F32, name="pm", tag="pm")
                        for tap in range(9):
                            dh, dw = tap // 3, tap % 3
                            rbase = (8 * m + dh) * WPAD + dw
                            nc.tensor.matmul(
                                pm[:],
                                w_sb[:, tap, co],
                                act[:, :, rbase : rbase + NPX],
                                start=(tap == 0),
                                stop=(tap == 8),
                                perf_mode=mybir.MatmulPerfMode.DoubleRowSwInterleave,
                            )
                        y = y_pool.tile([P, NPX], F32, name="y", tag="y")
                        # y = conv * scale + shift   (per-partition co consts)
                        nc.scalar.activation(
                            y[:], pm[:],
                            mybir.ActivationFunctionType.Identity,
                            bias=t_sb[:, co : co + 1],
                            scale=s_sb[:, co : co + 1],
                        )
                        y_tiles[(co, m)] = y

                # ---- transpose back to [px, co], fused relu, stage, store ----
                ev = out_ap[b].rearrange("(k two) w c -> w two k c", two=2)
                for m in range(G8):
                    for r in range(4):
                        k = 4 * m + r
                        pu = psu_pool.tile([116, COUT], F32)
                        for co in range(2):
                            nc.tensor.matmul(
                                pu[:, co * P : (co + 1) * P],
                                y_tiles[(co, m)][:, 116 * r : 116 * r + 116],
                                ident[:, :P],
                                is_transpose=True,
                                start=(co == 0),
                                stop=(co == 1),
                            )
                        # partitions: 0 pad | 1..56 row 2k | 57,58 pad |
                        # 59..114 row 2k+1 | 115 pad
                        nc.vector.tensor_scalar(
                            ub[:116, k, :], pu[:], 0.0, None, op0=AluOp.max
                        )
                    # store the 8 finished rows (4 row-pairs) of this group,
                    # split across the two DGE paths so issue overlaps.
                    ksl = slice(4 * m, 4 * m + 4)
                    nc.gpsimd.dma_start(ev[:, 0, ksl], ub[1:57, ksl])
                    nc.gpsimd.dma_start(ev[:, 1, ksl], ub[59:115, ksl])

    nc.compile()
    return nc


_NC_CACHE = None


def _get_nc():
    global _NC_CACHE
    if _NC_CACHE is None:
        _NC_CACHE = _build_program()
    return _NC_CACHE


def _prep_inputs(x, bias1, kernel, bn_beta, bn_mean, bn_var, bias2):
    x = np.asarray(x, dtype=np.float32)
    bias1 = np.asarray(bias1, dtype=np.float32)
    kernel = np.asarray(kernel, dtype=np.float32)
    bn_beta = np.asarray(bn_beta, dtype=np.float32)
    bn_mean = np.asarray(bn_mean, dtype=np.float32)
    bn_var = np.asarray(bn_var, dtype=np.float32)
    bias2 = np.asarray(bias2, dtype=np.float32).reshape(-1)

    # Weights: ste_sign with sign(0)=+1, exact in fp8e4.
    # DoubleRowSwInterleave stationary layout per (tap, co_t):
    # [A127 B127 A126 B126 ... A0 B0] where A/B are the ci halves
    # (ci = o*128 + ki, matching the act buffer's [ki, ci_t, px] pairing)
    # and columns are stored co-reversed.
    wq = np.where(kernel >= 0, np.float32(1.0), np.float32(-1.0))
    wq = wq.reshape(9, 2, P, 2, P)[..., ::-1]       # [tap, o, ki, co_t, m]
    wq = wq.transpose(2, 0, 3, 4, 1)                # [ki, tap, co_t, m, o]
    wq = np.ascontiguousarray(wq).reshape(P, 9, 2, 2 * P)
    wq = wq.astype(ml_dtypes.float8_e4m3)

    s = (1.0 / np.sqrt(bn_var + np.float32(EPS))).astype(np.float32)
    t = (bn_beta - bn_mean * s + bias2).astype(np.float32)
    # activations are encoded as sign/2, so the conv yields y/2: scale by 2s.
    s2 = (2.0 * s).astype(np.float32)

    in_maps = []
    for c in range(N_CORES):
        in_maps.append(
            {
                "x": np.ascontiguousarray(x[c * IMG : (c + 1) * IMG]),
                "wq": wq,
                "b1": np.ascontiguousarray(bias1.reshape(-1)),
                "s": np.ascontiguousarray(s2.reshape(2, P)),
                "t": np.ascontiguousarray(t.reshape(2, P)),
            }
        )
    return in_maps


def _ensure_ntff_hook():
    """This container ships the NTFF profiling machinery but not the
    ``antenv.axon_hooks`` shim module bass_utils imports it through;
    synthesize it so trace=True can capture HW exec times."""
    import sys
    import types

    if "antenv.axon_hooks" in sys.modules:
        return
    import antenv
    from trn_agent_boot.trn_boot import _ntff_profile_via_ctypes

    hook = _ntff_profile_via_ctypes("/opt/axon/libaxon_pjrt.so")
    mod = types.ModuleType("antenv.axon_hooks")
    mod.get_axon_ntff_profile_hook = lambda: hook
    mod.set_axon_ntff_profile_hook = lambda h: None
    sys.modules["antenv.axon_hooks"] = mod
    antenv.axon_hooks = mod


def run(inputs: dict, trace: bool = False):
    """Run the SPMD kernel. Returns (out, exec_time_ns or None)."""
    nc = _get_nc()
    in_maps = _prep_inputs(**inputs)
    if trace:
        try:
            _ensure_ntff_hook()
        except Exception as e:  # degrade to untraced run
            print(f"ntff hook unavailable: {e}")
    res = run_bass_kernel_spmd(
        nc, in_maps, core_ids=list(range(N_CORES)), trace=trace
    )
    out = np.concatenate([r["out"] for r in res.results], axis=0)
    return out, res.exec_time_ns


def kernel(**inputs) -> np.ndarray:
    out, _ = run(inputs, trace=False)
    return out
